# revision 36
# baseline (speedup 1.0000x reference)
"""Multi-head causal self-attention (b=4, n=2048, d=1024, 16 heads) on 8 TRN2 cores.

Sharding: core i handles batch b = i//2 and head-group g = i%2 (8 heads each).
Per core: QKV projections for its head-group, flash-style causal attention in
scoresT [k, q] layout (bf16 matmuls, fp32 PSUM accum, head pairs packed onto
the PE array via tile_position), and a partial transposed output projection.
Host: out[b] = (outT[2b] + outT[2b+1]).T + bo.
"""
import sys

for _p in ("/opt/trn_rl_repo", "/root/.axon_site", "/root/.axon_site/_ro/trn_rl_repo",
           "/root/.axon_site/_ro/pypackages"):
    if _p not in sys.path:
        sys.path.append(_p)

import os as _os
import json as _json
import tempfile as _tempfile

import numpy as np
import ml_dtypes


def _pin_act_tables():
    if _os.environ.get("KERNEL_NO_ACT_PIN"):
        return
    """Force walrus to place exp and ln in the one table set that holds both
    (natural_log_exp_and_others), so interleaved softmax-exp and 1/x=exp(-ln x)
    never thrash ACT_TABLE_LOADs."""
    if _os.environ.get("BASS_ACT_ROOT_JSON_PATH"):
        return
    import neuronxcc
    root = _os.path.join(_os.path.dirname(neuronxcc.__file__),
                         "pwp", "pwp_bin_trainium")
    src = _os.path.join(root, "act_info.json")
    if not _os.path.exists(src):
        return
    j = _json.load(open(src))
    for st in j.get("act_func_sets", []):
        if st.get("name") != "natural_log_exp_and_others":
            st.get("act", {}).pop("exp", None)
            st.get("act", {}).pop("ln", None)
    d = _tempfile.mkdtemp(prefix="act_root_")
    dst = _os.path.join(d, "act_info.json")
    # table .bin files are referenced relative to the json's directory
    for f in _os.listdir(root):
        if f != "act_info.json":
            try:
                _os.symlink(_os.path.join(root, f), _os.path.join(d, f))
            except OSError:
                pass
    _json.dump(j, open(dst, "w"))
    _os.environ["BASS_ACT_ROOT_JSON_PATH"] = dst


_pin_act_tables()

import concourse.bass as bass
import concourse.hw_specs as _hw_specs
import concourse.bacc as bacc
from concourse import mybir

# bacc's act-table-load placement reads the stock act_info.json via
# get_activation_tables; restrict exp/ln to the shared set there too so the
# softmax exp and the 1/x = exp(-ln x) chain never switch ACT table sets.
_orig_gat = _hw_specs.get_activation_tables


def _gat_pinned(arch):
    d = _orig_gat(arch)
    if _os.environ.get("KERNEL_NO_ACT_PIN"):
        return d
    for name, funcs in d.items():
        if name != "natural_log_exp_and_others":
            funcs.discard(mybir.ActivationFunctionType.Exp)
            funcs.discard(mybir.ActivationFunctionType.Ln)
    return d


_hw_specs.get_activation_tables = _gat_pinned
bacc.get_activation_tables = _gat_pinned
import concourse.bacc as bacc
import concourse.tile as tile
from concourse import mybir
from concourse.bass_utils import run_bass_kernel_spmd

BF16 = mybir.dt.bfloat16
F32 = mybir.dt.float32

N = 2048          # sequence length
D_IN = 1024       # model dim
D_LOC = 512       # per-core head-group width (8 heads * 64)
NPAIRS = 4        # head pairs per core
NT = 4            # q tiles of 512
SCALE = 1.0 / 8.0  # 1/sqrt(head_dim)

EXP = mybir.ActivationFunctionType.Exp
LOG = mybir.ActivationFunctionType.Ln
MULT = mybir.AluOpType.mult


def _build_program():
    nc = bacc.Bacc("TRN2", target_bir_lowering=False, debug=False, num_devices=8)

    xT = nc.dram_tensor("xT", [D_IN, N], BF16, kind="ExternalInput").ap()
    wq = nc.dram_tensor("wq", [D_IN, D_LOC], BF16, kind="ExternalInput").ap()
    wk = nc.dram_tensor("wk", [D_IN, D_LOC], BF16, kind="ExternalInput").ap()
    wv = nc.dram_tensor("wv", [D_IN, D_LOC], BF16, kind="ExternalInput").ap()
    wo = nc.dram_tensor("wo", [D_LOC, D_IN], BF16, kind="ExternalInput").ap()
    masks = nc.dram_tensor("masks", [128, 128], BF16, kind="ExternalInput").ap()
    outT = nc.dram_tensor("outT", [D_IN, N], F32, kind="ExternalOutput").ap()

    with tile.TileContext(nc) as tc:
        with tc.tile_pool(name="persist", bufs=1) as pp, \
             tc.tile_pool(name="qkv", bufs=1) as qkvp, \
             tc.tile_pool(name="exp", bufs=4) as ep, \
             tc.tile_pool(name="small", bufs=3) as sp, \
             tc.tile_pool(name="evac", bufs=3) as evp, \
             tc.tile_pool(name="ps_s", bufs=2, space="PSUM") as ps_s_pool, \
             tc.tile_pool(name="ps_c", bufs=1, space="PSUM") as ps_c_pool, \
             tc.tile_pool(name="ps_m", bufs=1, space="PSUM") as ps_m_pool, \
             tc.tile_pool(name="ps_p", bufs=2, space="PSUM") as ps_p_pool:

            # ---- persistent SBUF loads ----
            # order matters: the first compute (V projection chunk c, k-chunk
            # kk) needs wv[kk] + xT[kk]; interleave them so deps land early.
            mask_sb = pp.tile([128, 128], BF16, tag="masks")
            nc.sync.dma_start(mask_sb[:], masks[:])
            xT_sb, w_sb = [], {"wq": [], "wk": [], "wv": []}
            for i in range(8):
                t = pp.tile([128, D_LOC], BF16, tag=f"wv{i}", name=f"wv{i}")
                nc.sync.dma_start(t[:], wv[i * 128:(i + 1) * 128, :])
                w_sb["wv"].append(t)
                t = pp.tile([128, N], BF16, tag=f"xT{i}", name=f"xT{i}")
                nc.sync.dma_start(t[:], xT[i * 128:(i + 1) * 128, :])
                xT_sb.append(t)
            for name, src in (("wq", wq), ("wk", wk)):
                for i in range(8):
                    t = pp.tile([128, D_LOC], BF16, tag=f"{name}{i}", name=f"{name}{i}")
                    nc.sync.dma_start(t[:], src[i * 128:(i + 1) * 128, :])
                    w_sb[name].append(t)
            wo_sb = []
            for i in range(4):
                t = pp.tile([128, D_IN], BF16, tag=f"wo{i}", name=f"wo{i}")
                nc.sync.dma_start(t[:], wo[i * 128:(i + 1) * 128, :])
                wo_sb.append(t)

            ones64 = pp.tile([128, 64], BF16, tag="ones64")
            nc.vector.memset(ones64[:], 1.0)

            ctxT_sb = [pp.tile([128, N], BF16, tag=f"ctxT{p}", name=f"ctxT{p}")
                       for p in range(NPAIRS)]
            sums_sb = [pp.tile([128, N], F32, tag=f"sums{p}", name=f"sums{p}")
                       for p in range(NPAIRS)]
            invh_sb = [pp.tile([128, N], BF16, tag=f"invh{p}", name=f"invh{p}")
                       for p in range(NPAIRS)]

            # ---- V projection (full-width N=512 matmuls), emitted lazily:
            # attention(0, t) only needs chunks 0..4t+3, so later chunks are
            # threaded through pair-0's attention as filler.
            # v_all[r, c*512 + d] = v[c*128 + r, d]
            v_all = pp.tile([128, 16 * 512], BF16, tag="v_all")

            def vproj(c):
                ps = ps_p_pool.tile([128, 512], F32, tag="proj", name="ps_v")
                for kk in range(8):
                    nc.tensor.matmul(
                        ps[:], xT_sb[kk][:, c * 128:(c + 1) * 128],
                        w_sb["wv"][kk][:],
                        start=(kk == 0), stop=(kk == 7))
                nc.scalar.copy(v_all[:, c * 512:(c + 1) * 512], ps[:])

            for c in range(4):
                vproj(c)

            qT_sb, kT_sb = [], []

            def qkproj(p):
                pc = slice(p * 128, (p + 1) * 128)
                qT = qkvp.tile([128, N], BF16, tag=f"qT{p}", name=f"qT{p}")
                kT = qkvp.tile([128, N], BF16, tag=f"kT{p}", name=f"kT{p}")
                for dst, wname in ((qT, "wq"), (kT, "wk")):
                    for n in range(NT):
                        ps = ps_p_pool.tile([128, 512], F32, tag="proj", name="ps_qk")
                        for kk in range(8):
                            nc.tensor.matmul(
                                ps[:], w_sb[wname][kk][:, pc],
                                xT_sb[kk][:, n * 512:(n + 1) * 512],
                                start=(kk == 0), stop=(kk == 7))
                        nc.vector.tensor_copy(dst[:, n * 512:(n + 1) * 512], ps[:])
                qT_sb.append(qT)
                kT_sb.append(kT)

            def vsl(p, j, h):
                return v_all[:, j * 512 + p * 128 + h * 64:
                             j * 512 + p * 128 + (h + 1) * 64]

            def attention(p, t):
                qT, kT = qT_sb[p], kT_sb[p]
                nkc = 4 * t + 4
                ps_c = ps_c_pool.tile([128, 512], F32, tag="ctx", name="ps_c")
                ps_m = ps_m_pool.tile([128, 512], F32, tag="mb", name="ps_m")
                e_prev = None
                for j in range(nkc):
                    last = (j == nkc - 1)
                    bnd = (j // 4 == t)
                    o = j % 4
                    q0 = o * 128 if bnd else 0
                    qs = slice(t * 512 + q0, (t + 1) * 512)
                    ps_sc = ps_s_pool.tile([128, 1024], F32, tag="scores", name="ps_sc")
                    nc.tensor.matmul(
                        ps_sc[:, q0:512], kT[0:64, j * 128:(j + 1) * 128],
                        qT[0:64, qs],
                        start=True, stop=True, tile_position=(0, 0))
                    nc.tensor.matmul(
                        ps_sc[:, 512 + q0:1024], kT[64:128, j * 128:(j + 1) * 128],
                        qT[64:128, qs],
                        start=True, stop=True, tile_position=(64, 0))
                    e = ep.tile([128, 1024], BF16, tag="e", name="e")
                    if bnd:
                        src = ps_sc[:].rearrange("p (c w) -> p c w", c=2)[:, :, q0:512]
                        dst = e[:].rearrange("p (c w) -> p c w", c=2)[:, :, q0:512]
                        nc.scalar.activation(dst, src, EXP, scale=SCALE)
                        mdst = e[:].rearrange("p (c w) -> p c w", c=2)[:, :, q0:q0 + 128]
                        msrc = mask_sb[:]
                        msrc2 = bass.AP(msrc.tensor, msrc.offset,
                                        [list(msrc.ap[0]), [0, 2], [1, 128]])
                        nc.vector.tensor_tensor(mdst, mdst, msrc2, op=MULT)
                    else:
                        nc.scalar.activation(e[:], ps_sc[:], EXP, scale=SCALE)
                    nc.tensor.matmul(
                        ps_c[0:64, q0:512], vsl(p, j, 0), e[:, q0:512],
                        start=(j == 0), stop=last, tile_position=(0, 0))
                    nc.tensor.matmul(
                        ps_c[64:128, q0:512], vsl(p, j, 1), e[:, 512 + q0:1024],
                        start=(j == 0), stop=last, tile_position=(0, 64))
                    # softmax denominators: full chunks accumulate into one
                    # bf16 running total on DVE (rounding errors cancel in the
                    # 128-way reduction); the PE sees one ones-matmul pair for
                    # the whole full region plus one per boundary chunk.
                    if not bnd:
                        if j == 0:
                            e_tot = ep.tile([128, 1024], BF16, tag="e2", name="e_tot")
                            nc.vector.tensor_copy(e_tot[:], e[:])
                        else:
                            nc.vector.tensor_tensor(e_tot[:], e_tot[:], e[:],
                                                    op=mybir.AluOpType.add)
                        if j == 4 * t - 1:  # last full chunk: reduce the total
                            nc.tensor.matmul(
                                ps_m[0:64, :], ones64[:, :], e_tot[:, 0:512],
                                start=True, stop=False, tile_position=(0, 0))
                            nc.tensor.matmul(
                                ps_m[64:128, :], ones64[:, :], e_tot[:, 512:1024],
                                start=True, stop=False, tile_position=(0, 64))
                    else:
                        start_sums = (j == 0)  # t == 0 only: no full chunks
                        nc.tensor.matmul(
                            ps_m[0:64, q0:512], ones64[:, :], e[:, q0:512],
                            start=start_sums, stop=last, tile_position=(0, 0))
                        nc.tensor.matmul(
                            ps_m[64:128, q0:512], ones64[:, :], e[:, 512 + q0:1024],
                            start=start_sums, stop=last, tile_position=(0, 64))
                ts_ = slice(t * 512, (t + 1) * 512)
                nc.vector.tensor_copy(ctxT_sb[p][:, ts_], ps_c[:])
                nc.vector.tensor_copy(sums_sb[p][:, ts_], ps_m[:])

            def normalize_pair(p):
                # 1/s = exp(-ln s) on ACT; sums arrive pre-broadcast on all rows
                nc.scalar.activation(sums_sb[p][:, :], sums_sb[p][:, :], LOG)
                nc.scalar.activation(invh_sb[p][:, :], sums_sb[p][:, :],
                                     EXP, scale=-1.0)
                for t in range(NT):
                    ts_ = slice(t * 512, (t + 1) * 512)
                    nc.vector.tensor_tensor(
                        ctxT_sb[p][:, ts_], ctxT_sb[p][:, ts_],
                        invh_sb[p][:, ts_], op=MULT)

            def outproj_n(n):
                for m in range(8):
                    ps = ps_p_pool.tile([128, 512], F32, tag="proj", name="ps_out")
                    for p in range(NPAIRS):
                        nc.tensor.matmul(
                            ps[:], wo_sb[p][:, m * 128:(m + 1) * 128],
                            ctxT_sb[p][:, n * 512:(n + 1) * 512],
                            start=(p == 0), stop=(p == 3))
                    osb = evp.tile([128, 512], F32, tag="osb", name="osb")
                    nc.vector.tensor_copy(osb[:], ps[:])
                    nc.sync.dma_start(
                        outT[m * 128:(m + 1) * 128, n * 512:(n + 1) * 512], osb[:])

            def normalize_pt(p, t):
                # granular per-tile normalize (table-set pin keeps ACT loads at 1)
                ts_ = slice(t * 512, (t + 1) * 512)
                nc.scalar.activation(sums_sb[p][:, ts_], sums_sb[p][:, ts_], LOG)
                nc.scalar.activation(invh_sb[p][:, ts_], sums_sb[p][:, ts_],
                                     EXP, scale=-1.0)
                nc.vector.tensor_tensor(
                    ctxT_sb[p][:, ts_], ctxT_sb[p][:, ts_],
                    invh_sb[p][:, ts_], op=MULT)

            LAST = NPAIRS - 1
            for p in range(LAST):
                qkproj(p)
                for t in range(NT):
                    attention(p, t)
                    if p == 0 and t < NT - 1:
                        for c in range(4 * (t + 1), 4 * (t + 2)):
                            vproj(c)
                normalize_pair(p)
            qkproj(LAST)
            for t in reversed(range(NT)):
                attention(LAST, t)
                normalize_pt(LAST, t)
                outproj_n(t)

    nc.compile()
    return nc


_NC = None


def _get_program():
    global _NC
    if _NC is None:
        _NC = _build_program()
    return _NC


def _make_masks():
    r = np.arange(128)[:, None]
    c = np.arange(128)[None, :]
    return (c >= r).astype(ml_dtypes.bfloat16)


def kernel(inputs, Wq, Wk, Wv, Wo, bo):
    inputs = np.asarray(inputs, dtype=np.float32)
    Wq = np.asarray(Wq, dtype=np.float32)
    Wk = np.asarray(Wk, dtype=np.float32)
    Wv = np.asarray(Wv, dtype=np.float32)
    Wo = np.asarray(Wo, dtype=np.float32)
    bo = np.asarray(bo, dtype=np.float32)

    nc = _get_program()
    bf = ml_dtypes.bfloat16
    masks_np = _make_masks()

    in_maps = []
    for core in range(8):
        b, g = core // 2, core % 2
        gs = slice(g * D_LOC, (g + 1) * D_LOC)
        in_maps.append({
            "xT": np.ascontiguousarray(inputs[b].T).astype(bf),
            "wq": Wq[:, gs].astype(bf),
            "wk": Wk[:, gs].astype(bf),
            "wv": Wv[:, gs].astype(bf),
            "wo": np.ascontiguousarray(Wo[gs, :]).astype(bf),
            "masks": masks_np,
        })

    res = None
    for attempt in range(3):
        try:
            res = run_bass_kernel_spmd(nc, in_maps, core_ids=list(range(8)))
            break
        except Exception:
            if attempt == 2:
                raise
            import time as _time
            _time.sleep(5.0)
    out = np.empty((4, N, D_IN), dtype=np.float32)
    for b in range(4):
        acc = res.results[2 * b]["outT"] + res.results[2 * b + 1]["outT"]
        out[b] = acc.T + bo[None, :]
    return out
